# revision 11
# baseline (speedup 1.0000x reference)
"""Fused cross-attention kernel for Trainium2, data-parallel over batch on 8 cores.

Uses the low-rank structure of cross-attention (L=77 << D=512) plus token-mask
compaction. The text-side factors are folded on the host:
  tn  = LayerNorm(text)                 (exact, incl. beta)
  keep only unmasked tokens (count <= Lc = max over batch), then
  W2  = [Wq_h @ K_h^T]_h   [512, H*Lc]  (K = (tn @ Wk)[kept])
  W3  = [V_h @ Wo_h]_h     [H*Lc, 512]  (V = (tn @ Wv)[kept])
with zero padding up to Lc per head; padded score cols give exp(0)=1 which the
denominator correction (negcnt = count - Lc per head) removes exactly, and
padded W3 rows are zero so they add nothing to the output.

Per core (one batch element) the device runs, per 128-query tile:
  S    = X @ W2              [128, HL]   (4 fp8 MMs; fp8 halves input DMA)
  E    = exp(S * scale)                  (no max-sub: |S*scale| < 3)
  A    = E / (rowsum_h(E) + negcnt_h)    (masked-softmax via count fix)
  A^T  = PE-transpose in ceil(HL/128) chunks
  out  = sum_c A^T_c.T @ W3_c [128, 512] (bf16 accumulating MMs)

The softmax chain (exp / reduce / +negcnt / recip / scale) is batched over
TRIPLES of tiles: scores for 3 tiles accumulate into one PSUM bank
([128, 3, 164] f32 = 1968B/partition), so each ACT/DVE/GpSimd op runs once
per 3 tiles and the fixed per-op init costs (~185ns ACT, ~125ns DVE,
~95ns GpSimd) amortize 3x. LA=6 tiles of lookahead hides the longer chain
latency from the PE. The PSUM->SBUF output cast is split ACT/DVE to balance
engine busy time; attnT copies run on DVE (bf16 2x mode).

DMA layouts give >=2KB contiguous runs per descriptor:
  xt  DRAM [128, 4, NQ] fp8    (group loads of 16 tiles -> 2KB runs, sync queue)
  out DRAM [128, 98, 512] bf16 (8-tile stores -> 8KB runs, gpsimd queue so
                                store sem-waits don't block load issues);
host untransposes the output.
"""

import math
import sys

sys.path.insert(0, "/opt/trn_rl_repo")

import numpy as np
import ml_dtypes

import concourse.bass as bass
import concourse.mybir as mybir
import concourse.tile as tile
from concourse import bacc
from concourse.bass_utils import run_bass_kernel_spmd
from concourse.masks import make_identity

N_CORES = 8
B, T, S_, D, L, H = 8, 64, 196, 512, 77, 4
DH = D // H  # 128
NQ = T * S_  # 12544
LN_EPS = 1e-6
SCALE = float(DH) ** -0.5
P = 128
NCH = D // P  # 4 chunks of the feature dim

F32 = mybir.dt.float32
BF16 = mybir.dt.bfloat16
F8E4 = mybir.dt.float8e4

W2SCALE = 8.0  # host multiplies W2 by this pre-fp8-cast; exp scale divides it out
SPL = 384  # out-copy split point: ACT copies [0:SPL], DVE copies [SPL:D]
TB = 3  # softmax-chain tile batch (3 * 656B = 1968B fits one PSUM bank)
OB = 8  # tiles per output store

LAST_RESULTS = None  # BassKernelResults of the most recent run (for test harness)
_PROGRAM_CACHE = {}


def _input_groups(ntiles):
    """Ramped group sizes so PE never starves while the DMA pipe fills."""
    groups = []
    t0 = 0
    for gt in (2, 4, 8):
        if t0 + gt > ntiles:
            break
        groups.append((t0, gt))
        t0 += gt
    while t0 < ntiles:
        gt = min(16, ntiles - t0)
        groups.append((t0, gt))
        t0 += gt
    return groups


def build_program(nq=NQ, lc=41):
    """One SPMD program; all 8 cores run it on their own batch element."""
    hl = H * lc
    ncc = (hl + P - 1) // P  # chunks of the compacted (head, token) dim
    nt = nq // P  # 98 query tiles

    nc = bacc.Bacc("TRN2", target_bir_lowering=False, debug=False, num_devices=N_CORES)

    xt = nc.dram_tensor("xt", [P, NCH, nq], F8E4, kind="ExternalInput").ap()
    w2 = nc.dram_tensor("w2", [P, NCH, hl], F8E4, kind="ExternalInput").ap()
    w3 = nc.dram_tensor("w3", [P, ncc, D], BF16, kind="ExternalInput").ap()
    negcnt = nc.dram_tensor("negcnt", [P, 1], F32, kind="ExternalInput").ap()
    out = nc.dram_tensor("out", [P, nt, D], BF16, kind="ExternalOutput").ap()

    igroups = _input_groups(nt)
    group_of = {}
    for gi, (t0, gt) in enumerate(igroups):
        for t in range(t0, t0 + gt):
            group_of[t] = gi

    def tstart(t):  # softmax-chain triple containing tile t
        return (t // TB) * TB

    def tsize(t):
        return min(TB, nt - tstart(t))

    # output store groups (8KB runs); tail may be smaller
    ogroups = [(s, min(OB, nt - s)) for s in range(0, nt, OB)]
    ogstart = {s: (s, g) for s, g in ogroups}
    ogend = {s + g - 1: (s, g) for s, g in ogroups}

    with tile.TileContext(nc) as tc:
        with (
            tc.tile_pool(name="const", bufs=1) as const,
            tc.tile_pool(name="xtp", bufs=3) as xtp,
            tc.tile_pool(name="attp", bufs=4) as attp,
            tc.tile_pool(name="attnTp", bufs=3) as attnTp,
            tc.tile_pool(name="smalls", bufs=8) as smalls,
            tc.tile_pool(name="outp", bufs=2) as outp,
            tc.tile_pool(name="ps_sc", bufs=2, space="PSUM") as ps_sc,
            tc.tile_pool(name="ps_at", bufs=2, space="PSUM") as ps_at,
            tc.tile_pool(name="ps_out", bufs=2, space="PSUM") as ps_out,
        ):
            # ---- prolog loads; w2 + first xt groups gate the first matmul ----
            w2_sb = const.tile([P, NCH, hl], F8E4, tag="w2")
            nc.sync.dma_start(out=w2_sb[:], in_=w2)

            xt_sbs = {}

            def load_group(gi):
                t0, gt = igroups[gi]
                xt_sb = xtp.tile([P, NCH, gt * P], F8E4, tag="xt", name="xt_sb")
                nc.sync.dma_start(out=xt_sb[:], in_=xt[:, :, t0 * P : (t0 + gt) * P])
                xt_sbs[gi] = xt_sb

            load_group(0)
            load_group(1)

            w3_sb = const.tile([P, ncc, D], BF16, tag="w3")
            nc.scalar.dma_start(out=w3_sb[:], in_=w3)
            negcnt_sb = const.tile([P, 1], F32, tag="negcnt")
            nc.scalar.dma_start(out=negcnt_sb[:], in_=negcnt)

            ident = const.tile([P, P], BF16)
            make_identity(nc, ident)

            # PE warm-up: real matmuls (transpose-mode does not count as
            # PE-busy for the HAM clock gate) keep the PE busy through the
            # DMA head so HAM reaches 8/8 before real work starts.
            for _ in range(28):
                warm = ps_sc.tile([P, TB, hl], F32, tag="ps_s", name="warm")
                nc.tensor.matmul(
                    warm[:, 0, :P], ident[:], ident[:], start=True, stop=True
                )

            # ---- main loop, software-pipelined ----
            # The PE executes matmuls strictly in program order, so the score
            # matmuls are emitted LA tiles ahead of the same tile's transposes:
            # by the time the PE reaches transpose(j), the triple-batched
            # ACT/DVE/GpSimd softmax chain for tile j has had LA tile-periods
            # to finish. Out matmuls trail the transposes by one tile; the
            # paired PSUM->SBUF out copies (ACT/DVE split) trail two.
            LA = 9
            trip_ps_s = {}
            trip_attn = {}
            trip_ps_a = {}
            trip_attnT = {}
            pair_ps_o = {}
            cur_ob = None
            cur_os = 0
            for i in range(nt + LA + 6):
                if i < nt:
                    gi = group_of[i]
                    g0, ggt = igroups[gi]
                    if i == g0 and gi + 2 < len(igroups):
                        load_group(gi + 2)  # prefetch one group ahead
                    xt_sb = xt_sbs[gi]
                    tq = slice((i - g0) * P, (i - g0 + 1) * P)
                    tr = i // TB
                    ts, tsz = tstart(i), tsize(i)
                    if i == ts:
                        trip_ps_s[tr] = ps_sc.tile(
                            [P, tsz, hl], F32, tag="ps_s", name="ps_s"
                        )
                    ps_s = trip_ps_s[tr]
                    for kc in range(NCH):
                        nc.tensor.matmul(
                            ps_s[:, i - ts, :],
                            xt_sb[:, kc, tq],
                            w2_sb[:, kc, :],
                            start=(kc == 0),
                            stop=(kc == NCH - 1),
                        )
                    if i == ts + tsz - 1:
                        # whole-triple softmax chain: one op per engine
                        ps_s = trip_ps_s.pop(tr)
                        exp_sb = attp.tile([P, tsz, hl], BF16, tag="exp", name="exp")
                        nc.scalar.activation(
                            exp_sb[:], ps_s[:], mybir.ActivationFunctionType.Exp,
                            scale=SCALE / W2SCALE,
                        )
                        sumexp = smalls.tile([P, tsz * H], BF16, tag="sumexp")
                        with nc.allow_low_precision(
                            reason="bf16 rowsum of bf16 exp: 0.4% on Z, "
                            "enables DVE 2x mode"
                        ):
                            nc.vector.reduce_sum(
                                out=sumexp[:],
                                in_=exp_sb[:].rearrange("p t (h l) -> p (t h) l", h=H),
                                axis=mybir.AxisListType.X,
                            )
                        sumadj = smalls.tile([P, tsz * H], F32, tag="sumadj")
                        nc.vector.tensor_scalar_add(
                            sumadj[:], sumexp[:], negcnt_sb[:]
                        )
                        recip = smalls.tile([P, tsz * H], F32, tag="recip")
                        nc.vector.reciprocal_approx_fast(recip[:], sumadj[:])
                        attn_sb = attp.tile([P, tsz, hl], BF16, tag="attn", name="attn")
                        nc.gpsimd.tensor_mul(
                            attn_sb[:].rearrange("p t (h l) -> p (t h) l", h=H),
                            exp_sb[:].rearrange("p t (h l) -> p (t h) l", h=H),
                            recip[:].to_broadcast([P, tsz * H, lc]),
                        )
                        trip_attn[tr] = attn_sb

                # out matmuls trail by LA+3 (one past the whole-triple attnT
                # copy) and are SPLIT around the transpose section: the two
                # N=512 out matmuls are the only ones whose LDWEIGHTS fails to
                # hide when they are adjacent (both weight buffers stay busy),
                # so interleaving shorter matmuls between them restores the
                # LDW pull-ahead.
                m = i - LA - 4
                if 0 <= m < nt:
                    pr = m // 2
                    if m % 2 == 0:
                        pair_ps_o[pr] = ps_out.tile(
                            [P, 2, D], F32, tag="ps_o", name="ps_o"
                        )
                    # only the first matmul touching this psum tile may set
                    # start=True: start clears the whole bank's has_written
                    # bits, so a second start would flip the already-written
                    # half back to overwrite-mode downstream
                    for nh in range(2):
                        ncol = slice(nh * 256, (nh + 1) * 256)
                        nc.tensor.matmul(
                            pair_ps_o[pr][:, m % 2, ncol],
                            trip_attnT[m // TB][:P, m - tstart(m), 0, :],
                            w3_sb[:P, 0, ncol],
                            start=(nh == 0),
                            stop=False,
                            skip_group_check=True,
                        )

                j = i - LA
                if 0 <= j < nt:
                    tr = j // TB
                    ts, tsz = tstart(j), tsize(j)
                    if j == ts:
                        trip_ps_a[tr] = ps_at.tile(
                            [P, tsz, ncc * P], BF16, tag="ps_a", name="ps_a"
                        )
                    ps_a = trip_ps_a[tr]
                    attn_sb = trip_attn[tr]
                    for c in range(ncc):
                        cw = min(P, hl - c * P)
                        nc.tensor.transpose(
                            ps_a[:cw, j - ts, c * P : (c + 1) * P],
                            attn_sb[:, j - ts, c * P : c * P + cw],
                            ident[:],
                        )
                    if j == ts + tsz - 1:
                        trip_attn.pop(tr)
                        attnT = attnTp.tile(
                            [P, tsz, ncc, P], BF16, tag="attnT", name="attnT"
                        )
                        nc.vector.tensor_copy(
                            attnT[:].rearrange("p t c q -> p (t c q)"),
                            trip_ps_a.pop(tr)[:].rearrange("p t n -> p (t n)"),
                        )
                        trip_attnT[tr] = attnT


                if 0 <= m < nt:
                    cw = hl - P
                    for nh in range(2):
                        ncol = slice(nh * 256, (nh + 1) * 256)
                        nc.tensor.matmul(
                            pair_ps_o[m // 2][:, m % 2, ncol],
                            trip_attnT[m // TB][:cw, m - tstart(m), 1, :],
                            w3_sb[:cw, 1, ncol],
                            start=False,
                            stop=(nh == 1),
                            skip_group_check=True,
                        )
                    if m == tstart(m) + tsize(m) - 1:
                        trip_attnT.pop(m // TB)

                if i >= nt:
                    # drain phase: the per-iteration PE work halves, which
                    # lets the HAM activity window go idle and re-throttle
                    # the clock to 4/8. Filler matmuls keep it at 8/8.
                    for _ in range(3):
                        fill = ps_sc.tile([P, TB, hl], F32, tag="ps_s", name="fill")
                        nc.tensor.matmul(
                            fill[:, 0, :P], ident[:], ident[:], start=True, stop=True
                        )

                k = i - LA - 5
                if 0 <= k < nt:
                    if k in ogstart:
                        s, g = ogstart[k]
                        cur_ob = outp.tile([P, g, D], BF16, tag="out", name="ob")
                        cur_os = s
                    if k % 2 == 1:
                        pr = k // 2
                        ps_o = pair_ps_o.pop(pr)
                        so = k - 1 - cur_os
                        # split the PSUM->SBUF cast: ACT does [0:SPL],
                        # DVE does [SPL:D] -- balances engine busy time
                        nc.scalar.copy(
                            cur_ob[:, so : so + 2, 0:SPL], ps_o[:, :, 0:SPL]
                        )
                        nc.vector.tensor_copy(
                            cur_ob[:, so : so + 2, SPL:D], ps_o[:, :, SPL:D]
                        )
                    if k in ogend:
                        s, g = ogend[k]
                        # store on the gpsimd queue: keeps store sem-waits
                        # from blocking xt load issues on the sync queue
                        nc.gpsimd.dma_start(out=out[:, s : s + g, :], in_=cur_ob[:])

    nc.compile()
    return nc


def _get_program(nq=NQ, lc=41):
    key = (nq, lc)
    if key not in _PROGRAM_CACHE:
        _PROGRAM_CACHE[key] = build_program(nq, lc)
    return _PROGRAM_CACHE[key]


def prep_core_inputs(visual_feat, text_feat, token_mask, wq, wk, wv, wo,
                     ln_gamma, ln_beta):
    """Host-side prep: shard over batch, compact masked tokens, fold the text
    side into W2/W3."""
    vf = np.ascontiguousarray(visual_feat.reshape(B, -1, D))
    nq = vf.shape[1]

    # Exact LayerNorm (f32, biased variance, incl. beta)
    mu = text_feat.mean(-1, keepdims=True)
    var = np.square(text_feat - mu).mean(-1, keepdims=True)
    tn = (text_feat - mu) / np.sqrt(var + LN_EPS) * ln_gamma + ln_beta  # [B, L, D]

    mask = np.asarray(token_mask).astype(bool)  # [B, L]
    counts = mask.sum(1)
    lc = int(counts.max())
    hl = H * lc
    ncc = (hl + P - 1) // P

    k_all = tn @ wk  # [B, L, D]
    v_all = tn @ wv
    wq4 = wq.reshape(D, H, DH)
    wo4 = wo.reshape(H, DH, D)

    in_maps = []
    for b in range(B):
        cnt = int(counts[b])
        kc = k_all[b][mask[b]].reshape(cnt, H, DH)  # [cnt, H, DH]
        vc = v_all[b][mask[b]].reshape(cnt, H, DH)
        # W2[d, h*lc+l] = sum_e Wq[d,(h,e)] K[l,(h,e)]
        w2_b = np.zeros((D, H, lc), np.float32)
        w2_b[:, :, :cnt] = np.einsum("dhe,lhe->dhl", wq4, kc, optimize=True)
        # W3[h*lc+l, d] = sum_e V[l,(h,e)] Wo[(h,e),d]
        w3_b = np.zeros((H, lc, D), np.float32)
        w3_b[:, :cnt, :] = np.einsum("lhe,hed->hld", vc, wo4, optimize=True)
        w3_pad = np.zeros((ncc * P, D), np.float32)
        w3_pad[:hl] = w3_b.reshape(hl, D)

        # xt DRAM layout [p, c, q]: xt[p, c, q] = X[q, c*128+p]
        # TRN FP8_EXP4 is e4m3 with max +-240 (256+ decodes as inf/nan)
        xt_c = vf[b].reshape(nq, NCH, P).transpose(2, 1, 0)
        xt_c = np.clip(xt_c, -240, 240).astype(ml_dtypes.float8_e4m3fn)
        xt_c = np.ascontiguousarray(xt_c)
        w2_c = (w2_b.reshape(D, hl) * W2SCALE).reshape(NCH, P, hl).transpose(1, 0, 2)
        w2_c = np.clip(w2_c, -240, 240).astype(ml_dtypes.float8_e4m3fn)
        w2_c = np.ascontiguousarray(w2_c)
        w3_c = np.ascontiguousarray(
            w3_pad.reshape(ncc, P, D).transpose(1, 0, 2).astype(ml_dtypes.bfloat16)
        )
        negcnt_b = np.full((P, 1), float(cnt - lc), np.float32)
        in_maps.append({
            "xt": xt_c,
            "w2": w2_c,
            "w3": w3_c,
            "negcnt": negcnt_b,
        })
    return in_maps, lc


def kernel(visual_feat, text_feat, token_mask, Wq, Wk, Wv, Wo, ln_gamma, ln_beta):
    global LAST_RESULTS
    visual_feat = np.asarray(visual_feat, np.float32)
    text_feat = np.asarray(text_feat, np.float32)
    token_mask = np.asarray(token_mask)

    in_maps, lc = prep_core_inputs(
        visual_feat, text_feat, token_mask,
        np.asarray(Wq, np.float32), np.asarray(Wk, np.float32),
        np.asarray(Wv, np.float32), np.asarray(Wo, np.float32),
        np.asarray(ln_gamma, np.float32), np.asarray(ln_beta, np.float32),
    )
    nc = _get_program(NQ, lc)
    res = run_bass_kernel_spmd(nc, in_maps, core_ids=list(range(N_CORES)))
    LAST_RESULTS = res
    # out DRAM layout is [p, tile, d]; query q = tile*128 + p
    out = np.stack(
        [
            np.ascontiguousarray(
                res.results[b]["out"].astype(np.float32).transpose(1, 0, 2)
            ).reshape(NQ, D)
            for b in range(B)
        ],
        axis=0,
    )
    return out.reshape(B, T, S_, D)


# revision 12
# speedup vs baseline: 1.0206x; 1.0206x over previous
"""Fused cross-attention kernel for Trainium2, data-parallel over batch on 8 cores.

Uses the low-rank structure of cross-attention (L=77 << D=512) plus token-mask
compaction. The text-side factors are folded on the host:
  tn  = LayerNorm(text)                 (exact, incl. beta)
  keep only unmasked tokens (count <= Lc = max over batch), then
  W2  = [Wq_h @ K_h^T]_h   [512, H*Lc]  (K = (tn @ Wk)[kept])
  W3  = [V_h @ Wo_h]_h     [H*Lc, 512]  (V = (tn @ Wv)[kept])
with zero padding up to Lc per head; padded score cols give exp(0)=1 which the
denominator correction (negcnt = count - Lc per head) removes exactly, and
padded W3 rows are zero so they add nothing to the output.

Per core (one batch element) the device runs, per 128-query tile:
  S    = X @ W2              [128, HL]   (4 fp8 MMs; fp8 halves input DMA)
  E    = exp(S * scale)                  (no max-sub: |S*scale| < 3)
  A    = E / (rowsum_h(E) + negcnt_h)    (masked-softmax via count fix)
  A^T  = PE-transpose in ceil(HL/128) chunks
  out  = sum_c A^T_c.T @ W3_c [128, 512] (bf16 accumulating MMs)

The softmax chain (exp / reduce / +negcnt / recip / scale) is batched over
TRIPLES of tiles: scores for 3 tiles accumulate into one PSUM bank
([128, 3, 164] f32 = 1968B/partition), so each ACT/DVE/GpSimd op runs once
per 3 tiles and the fixed per-op init costs (~185ns ACT, ~125ns DVE,
~95ns GpSimd) amortize 3x. LA=6 tiles of lookahead hides the longer chain
latency from the PE. The PSUM->SBUF output cast is split ACT/DVE to balance
engine busy time; attnT copies run on DVE (bf16 2x mode).

DMA layouts give >=2KB contiguous runs per descriptor:
  xt  DRAM [128, 4, NQ] fp8    (group loads of 16 tiles -> 2KB runs, sync queue)
  out DRAM [128, 98, 512] bf16 (8-tile stores -> 8KB runs, gpsimd queue so
                                store sem-waits don't block load issues);
host untransposes the output.
"""

import math
import sys

sys.path.insert(0, "/opt/trn_rl_repo")

import numpy as np
import ml_dtypes

import concourse.bass as bass
import concourse.mybir as mybir
import concourse.tile as tile
from concourse import bacc
from concourse.bass_utils import run_bass_kernel_spmd
from concourse.masks import make_identity

N_CORES = 8
B, T, S_, D, L, H = 8, 64, 196, 512, 77, 4
DH = D // H  # 128
NQ = T * S_  # 12544
LN_EPS = 1e-6
SCALE = float(DH) ** -0.5
P = 128
NCH = D // P  # 4 chunks of the feature dim

F32 = mybir.dt.float32
BF16 = mybir.dt.bfloat16
F8E4 = mybir.dt.float8e4

W2SCALE = 8.0  # host multiplies W2 by this pre-fp8-cast; exp scale divides it out
SPL = 384  # out-copy split point: ACT copies [0:SPL], DVE copies [SPL:D]
TB = 3  # softmax-chain tile batch (3 * 656B = 1968B fits one PSUM bank)
OB = 8  # tiles per output store

LAST_RESULTS = None  # BassKernelResults of the most recent run (for test harness)
_PROGRAM_CACHE = {}


def _input_groups(ntiles):
    """Ramped group sizes so PE never starves while the DMA pipe fills."""
    groups = []
    t0 = 0
    for gt in (2, 4, 8):
        if t0 + gt > ntiles:
            break
        groups.append((t0, gt))
        t0 += gt
    while t0 < ntiles:
        gt = min(16, ntiles - t0)
        groups.append((t0, gt))
        t0 += gt
    return groups


def build_program(nq=NQ, lc=41):
    """One SPMD program; all 8 cores run it on their own batch element."""
    hl = H * lc
    ncc = (hl + P - 1) // P  # chunks of the compacted (head, token) dim
    nt = nq // P  # 98 query tiles

    nc = bacc.Bacc("TRN2", target_bir_lowering=False, debug=False, num_devices=N_CORES)

    xt = nc.dram_tensor("xt", [P, NCH, nq], F8E4, kind="ExternalInput").ap()
    w2 = nc.dram_tensor("w2", [P, NCH, hl], F8E4, kind="ExternalInput").ap()
    w3 = nc.dram_tensor("w3", [P, ncc, D], BF16, kind="ExternalInput").ap()
    negcnt = nc.dram_tensor("negcnt", [P, 1], F32, kind="ExternalInput").ap()
    out = nc.dram_tensor("out", [P, nt, D], BF16, kind="ExternalOutput").ap()

    igroups = _input_groups(nt)
    group_of = {}
    for gi, (t0, gt) in enumerate(igroups):
        for t in range(t0, t0 + gt):
            group_of[t] = gi

    def tstart(t):  # softmax-chain triple containing tile t
        return (t // TB) * TB

    def tsize(t):
        return min(TB, nt - tstart(t))

    # output store groups (8KB runs); tail may be smaller
    ogroups = [(s, min(OB, nt - s)) for s in range(0, nt, OB)]
    ogstart = {s: (s, g) for s, g in ogroups}
    ogend = {s + g - 1: (s, g) for s, g in ogroups}

    with tile.TileContext(nc) as tc:
        with (
            tc.tile_pool(name="const", bufs=1) as const,
            tc.tile_pool(name="xtp", bufs=3) as xtp,
            tc.tile_pool(name="attp", bufs=4) as attp,
            tc.tile_pool(name="attnTp", bufs=3) as attnTp,
            tc.tile_pool(name="smalls", bufs=8) as smalls,
            tc.tile_pool(name="outp", bufs=2) as outp,
            tc.tile_pool(name="ps_sc", bufs=2, space="PSUM") as ps_sc,
            tc.tile_pool(name="ps_at", bufs=2, space="PSUM") as ps_at,
            tc.tile_pool(name="ps_out", bufs=2, space="PSUM") as ps_out,
        ):
            # ---- prolog loads; w2 + first xt groups gate the first matmul ----
            w2_sb = const.tile([P, NCH, hl], F8E4, tag="w2")
            nc.sync.dma_start(out=w2_sb[:], in_=w2)

            xt_sbs = {}

            def load_group(gi):
                t0, gt = igroups[gi]
                xt_sb = xtp.tile([P, NCH, gt * P], F8E4, tag="xt", name="xt_sb")
                nc.sync.dma_start(out=xt_sb[:], in_=xt[:, :, t0 * P : (t0 + gt) * P])
                xt_sbs[gi] = xt_sb

            load_group(0)
            load_group(1)

            w3_sb = const.tile([P, ncc, D], BF16, tag="w3")
            nc.scalar.dma_start(out=w3_sb[:], in_=w3)
            negcnt_sb = const.tile([P, 1], F32, tag="negcnt")
            nc.scalar.dma_start(out=negcnt_sb[:], in_=negcnt)

            ident = const.tile([P, P], BF16)
            make_identity(nc, ident)

            # PE warm-up: real matmuls (transpose-mode does not count as
            # PE-busy for the HAM clock gate) keep the PE busy through the
            # DMA head so HAM reaches 8/8 before real work starts.
            for _ in range(28):
                warm = ps_sc.tile([P, TB, hl], F32, tag="ps_s", name="warm")
                nc.tensor.matmul(
                    warm[:, 0, :P], ident[:], ident[:], start=True, stop=True
                )

            # ---- main loop, software-pipelined ----
            # The PE executes matmuls strictly in program order, so the score
            # matmuls are emitted LA tiles ahead of the same tile's transposes:
            # by the time the PE reaches transpose(j), the triple-batched
            # ACT/DVE/GpSimd softmax chain for tile j has had LA tile-periods
            # to finish. Out matmuls trail the transposes by one tile; the
            # paired PSUM->SBUF out copies (ACT/DVE split) trail two.
            LA = 7
            trip_ps_s = {}
            trip_attn = {}
            trip_ps_a = {}
            trip_attnT = {}
            pair_ps_o = {}
            ob_state = {"ob": None, "os": 0}

            def sec_c0(m):
                pr = m // 2
                if m % 2 == 0:
                    pair_ps_o[pr] = ps_out.tile(
                        [P, 2, D], F32, tag="ps_o", name="ps_o"
                    )
                # only the first matmul touching this psum tile may set
                # start=True: start clears the whole bank's has_written
                # bits, so a second start would flip the already-written
                # half back to overwrite-mode downstream
                for nh in range(2):
                    ncol = slice(nh * 256, (nh + 1) * 256)
                    nc.tensor.matmul(
                        pair_ps_o[pr][:, m % 2, ncol],
                        trip_attnT[m // TB][:P, m - tstart(m), 0, :],
                        w3_sb[:P, 0, ncol],
                        start=(nh == 0),
                        stop=False,
                        skip_group_check=True,
                    )

            def sec_j(j):
                tr = j // TB
                ts, tsz = tstart(j), tsize(j)
                if j == ts:
                    trip_ps_a[tr] = ps_at.tile(
                        [P, tsz, ncc * P], BF16, tag="ps_a", name="ps_a"
                    )
                ps_a = trip_ps_a[tr]
                attn_sb = trip_attn[tr]
                for c in range(ncc):
                    cw = min(P, hl - c * P)
                    nc.tensor.transpose(
                        ps_a[:cw, j - ts, c * P : (c + 1) * P],
                        attn_sb[:, j - ts, c * P : c * P + cw],
                        ident[:],
                    )
                if j == ts + tsz - 1:
                    trip_attn.pop(tr)
                    attnT = attnTp.tile(
                        [P, tsz, ncc, P], BF16, tag="attnT", name="attnT"
                    )
                    nc.vector.tensor_copy(
                        attnT[:].rearrange("p t c q -> p (t c q)"),
                        trip_ps_a.pop(tr)[:].rearrange("p t n -> p (t n)"),
                    )
                    trip_attnT[tr] = attnT

            def sec_c1(m):
                cw = hl - P
                for nh in range(2):
                    ncol = slice(nh * 256, (nh + 1) * 256)
                    nc.tensor.matmul(
                        pair_ps_o[m // 2][:, m % 2, ncol],
                        trip_attnT[m // TB][:cw, m - tstart(m), 1, :],
                        w3_sb[:cw, 1, ncol],
                        start=False,
                        stop=(nh == 1),
                        skip_group_check=True,
                    )
                if m == tstart(m) + tsize(m) - 1:
                    trip_attnT.pop(m // TB)

            def sec_k(k):
                if k in ogstart:
                    s, g = ogstart[k]
                    ob_state["ob"] = outp.tile([P, g, D], BF16, tag="out", name="ob")
                    ob_state["os"] = s
                cur_ob = ob_state["ob"]
                if k % 2 == 1:
                    pr = k // 2
                    ps_o = pair_ps_o.pop(pr)
                    so = k - 1 - ob_state["os"]
                    # split the PSUM->SBUF cast: ACT does [0:SPL],
                    # DVE does [SPL:D] -- balances engine busy time
                    nc.scalar.copy(
                        cur_ob[:, so : so + 2, 0:SPL], ps_o[:, :, 0:SPL]
                    )
                    nc.vector.tensor_copy(
                        cur_ob[:, so : so + 2, SPL:D], ps_o[:, :, SPL:D]
                    )
                if k in ogend:
                    s, g = ogend[k]
                    # store on the gpsimd queue: keeps store sem-waits
                    # from blocking xt load issues on the sync queue
                    nc.gpsimd.dma_start(
                        out=out[:, s : s + g, :], in_=cur_ob[:]
                    )

            # Trailing sections run at one tile per iteration mid-stream
            # (j = i - LA; out matmuls at j-3 SPLIT around the transposes so
            # their N=256 LDWEIGHTS can pull ahead; copies at j-4) and catch
            # up at two tiles per iteration once the scores are exhausted --
            # the softmax chains are all long done by then, and a short drain
            # keeps the HAM clock from re-throttling.
            jq = -LA
            i = 0
            while jq - 4 < nt:
                if i < nt:
                    gi = group_of[i]
                    g0, ggt = igroups[gi]
                    if i == g0 and gi + 2 < len(igroups):
                        load_group(gi + 2)  # prefetch one group ahead
                    xt_sb = xt_sbs[gi]
                    tq = slice((i - g0) * P, (i - g0 + 1) * P)
                    tr = i // TB
                    ts, tsz = tstart(i), tsize(i)
                    if i == ts:
                        trip_ps_s[tr] = ps_sc.tile(
                            [P, tsz, hl], F32, tag="ps_s", name="ps_s"
                        )
                    ps_s = trip_ps_s[tr]
                    for kc in range(NCH):
                        nc.tensor.matmul(
                            ps_s[:, i - ts, :],
                            xt_sb[:, kc, tq],
                            w2_sb[:, kc, :],
                            start=(kc == 0),
                            stop=(kc == NCH - 1),
                        )
                    if i == ts + tsz - 1:
                        # whole-triple softmax chain: one op per engine
                        ps_s = trip_ps_s.pop(tr)
                        exp_sb = attp.tile([P, tsz, hl], BF16, tag="exp", name="exp")
                        nc.scalar.activation(
                            exp_sb[:], ps_s[:], mybir.ActivationFunctionType.Exp,
                            scale=SCALE / W2SCALE,
                        )
                        sumexp = smalls.tile([P, tsz * H], BF16, tag="sumexp")
                        with nc.allow_low_precision(
                            reason="bf16 rowsum of bf16 exp: 0.4% on Z, "
                            "enables DVE 2x mode"
                        ):
                            nc.vector.reduce_sum(
                                out=sumexp[:],
                                in_=exp_sb[:].rearrange("p t (h l) -> p (t h) l", h=H),
                                axis=mybir.AxisListType.X,
                            )
                        sumadj = smalls.tile([P, tsz * H], F32, tag="sumadj")
                        nc.vector.tensor_scalar_add(
                            sumadj[:], sumexp[:], negcnt_sb[:]
                        )
                        recip = smalls.tile([P, tsz * H], F32, tag="recip")
                        nc.vector.reciprocal_approx_fast(recip[:], sumadj[:])
                        attn_sb = attp.tile([P, tsz, hl], BF16, tag="attn", name="attn")
                        nc.gpsimd.tensor_mul(
                            attn_sb[:].rearrange("p t (h l) -> p (t h) l", h=H),
                            exp_sb[:].rearrange("p t (h l) -> p (t h) l", h=H),
                            recip[:].to_broadcast([P, tsz * H, lc]),
                        )
                        trip_attn[tr] = attn_sb

                steps = 1 if i < nt else 2
                for _ in range(steps):
                    if jq - 4 >= nt:
                        break
                    m = jq - 3
                    if 0 <= m < nt:
                        sec_c0(m)
                    if 0 <= jq < nt:
                        sec_j(jq)
                    if 0 <= m < nt:
                        sec_c1(m)
                    k = jq - 4
                    if 0 <= k < nt:
                        sec_k(k)
                    jq += 1
                i += 1

    nc.compile()
    return nc


def _get_program(nq=NQ, lc=41):
    key = (nq, lc)
    if key not in _PROGRAM_CACHE:
        _PROGRAM_CACHE[key] = build_program(nq, lc)
    return _PROGRAM_CACHE[key]


def prep_core_inputs(visual_feat, text_feat, token_mask, wq, wk, wv, wo,
                     ln_gamma, ln_beta):
    """Host-side prep: shard over batch, compact masked tokens, fold the text
    side into W2/W3."""
    vf = np.ascontiguousarray(visual_feat.reshape(B, -1, D))
    nq = vf.shape[1]

    # Exact LayerNorm (f32, biased variance, incl. beta)
    mu = text_feat.mean(-1, keepdims=True)
    var = np.square(text_feat - mu).mean(-1, keepdims=True)
    tn = (text_feat - mu) / np.sqrt(var + LN_EPS) * ln_gamma + ln_beta  # [B, L, D]

    mask = np.asarray(token_mask).astype(bool)  # [B, L]
    counts = mask.sum(1)
    lc = int(counts.max())
    hl = H * lc
    ncc = (hl + P - 1) // P

    k_all = tn @ wk  # [B, L, D]
    v_all = tn @ wv
    wq4 = wq.reshape(D, H, DH)
    wo4 = wo.reshape(H, DH, D)

    in_maps = []
    for b in range(B):
        cnt = int(counts[b])
        kc = k_all[b][mask[b]].reshape(cnt, H, DH)  # [cnt, H, DH]
        vc = v_all[b][mask[b]].reshape(cnt, H, DH)
        # W2[d, h*lc+l] = sum_e Wq[d,(h,e)] K[l,(h,e)]
        w2_b = np.zeros((D, H, lc), np.float32)
        w2_b[:, :, :cnt] = np.einsum("dhe,lhe->dhl", wq4, kc, optimize=True)
        # W3[h*lc+l, d] = sum_e V[l,(h,e)] Wo[(h,e),d]
        w3_b = np.zeros((H, lc, D), np.float32)
        w3_b[:, :cnt, :] = np.einsum("lhe,hed->hld", vc, wo4, optimize=True)
        w3_pad = np.zeros((ncc * P, D), np.float32)
        w3_pad[:hl] = w3_b.reshape(hl, D)

        # xt DRAM layout [p, c, q]: xt[p, c, q] = X[q, c*128+p]
        # TRN FP8_EXP4 is e4m3 with max +-240 (256+ decodes as inf/nan)
        xt_c = vf[b].reshape(nq, NCH, P).transpose(2, 1, 0)
        xt_c = np.clip(xt_c, -240, 240).astype(ml_dtypes.float8_e4m3fn)
        xt_c = np.ascontiguousarray(xt_c)
        w2_c = (w2_b.reshape(D, hl) * W2SCALE).reshape(NCH, P, hl).transpose(1, 0, 2)
        w2_c = np.clip(w2_c, -240, 240).astype(ml_dtypes.float8_e4m3fn)
        w2_c = np.ascontiguousarray(w2_c)
        w3_c = np.ascontiguousarray(
            w3_pad.reshape(ncc, P, D).transpose(1, 0, 2).astype(ml_dtypes.bfloat16)
        )
        negcnt_b = np.full((P, 1), float(cnt - lc), np.float32)
        in_maps.append({
            "xt": xt_c,
            "w2": w2_c,
            "w3": w3_c,
            "negcnt": negcnt_b,
        })
    return in_maps, lc


def kernel(visual_feat, text_feat, token_mask, Wq, Wk, Wv, Wo, ln_gamma, ln_beta):
    global LAST_RESULTS
    visual_feat = np.asarray(visual_feat, np.float32)
    text_feat = np.asarray(text_feat, np.float32)
    token_mask = np.asarray(token_mask)

    in_maps, lc = prep_core_inputs(
        visual_feat, text_feat, token_mask,
        np.asarray(Wq, np.float32), np.asarray(Wk, np.float32),
        np.asarray(Wv, np.float32), np.asarray(Wo, np.float32),
        np.asarray(ln_gamma, np.float32), np.asarray(ln_beta, np.float32),
    )
    nc = _get_program(NQ, lc)
    res = run_bass_kernel_spmd(nc, in_maps, core_ids=list(range(N_CORES)))
    LAST_RESULTS = res
    # out DRAM layout is [p, tile, d]; query q = tile*128 + p
    out = np.stack(
        [
            np.ascontiguousarray(
                res.results[b]["out"].astype(np.float32).transpose(1, 0, 2)
            ).reshape(NQ, D)
            for b in range(B)
        ],
        axis=0,
    )
    return out.reshape(B, T, S_, D)
